# revision 1
# baseline (speedup 1.0000x reference)
"""Trainium2 Bass kernel for a 30-expert MLP ensemble.

Computes out[b] = mean_e sigmoid(relu(x @ W1[e] + b1[e]) @ W2[e] + b2[e])
for x [8192, 1024], W1 [30, 1024, 2048], W2 [30, 2048].

Strategy: data-parallel over the batch axis. Each of the 8 cores gets a
1024-row shard of x (pre-transposed on host) and the full replicated
weight stack. No collectives: the host concatenates the 8 disjoint
output shards. Matmuls run as float32r (full fp32 data, 1 cycle/row on
the PE when the moving free dim >= 256, i.e. 4x faster than plain fp32).

Layouts (prepared host-side in numpy):
  xt  [128, IB*BC]    xt[p, ib*BC + b]    = x[b, ib*128 + p]     (lhs^T shard)
  w1  [E, JB, 128, IB*128]  w1[e,jb,p,ib*128+q] = W1[e, ib*128+p, jb*128+q]
  b1  [128, E*JB]     b1[p, e*JB + jb]    = b1[e, jb*128 + p]
  w2  [128, E*JB]     w2[p, e*JB + jb]    = W2[e, jb*128 + p]
  b2  [1, E]

Per (expert e, hidden block jb): 16 fc1 matmuls accumulate
h^T[jb] = W1_blk^T x^T into PSUM [128, 512] (K = input dim, 8 blocks of
128); one Relu+bias activation evicts to SBUF; one fc2 matmul per batch
half (lhsT = W2 column [128, 1]) accumulates the per-expert logits in a
persistent PSUM [1, 512] across all 16 jb. Sigmoid+bias writes expert
row e of an SBUF tile o_all [30, BC]; a final ones(1/30) matmul reduces
over the expert partitions to the mean.
"""

import numpy as np

import concourse.bacc as bacc
import concourse.bass as bass
import concourse.mybir as mybir
import concourse.tile as tile
from concourse.bass_utils import run_bass_kernel_spmd

N_CORES = 8
P = 128
NB = 512  # matmul moving free dim (psum bank = 512 fp32)

E_FULL, I_FULL, H_FULL, B_FULL = 30, 1024, 2048, 8192


def build_bass(E=E_FULL, I=I_FULL, H=H_FULL, BC=B_FULL // N_CORES):
    IB = I // P
    JB = H // P
    BH = BC // NB
    f32 = mybir.dt.float32
    f32r = mybir.dt.float32r
    Relu = mybir.ActivationFunctionType.Relu
    Sigmoid = mybir.ActivationFunctionType.Sigmoid
    Copy = mybir.ActivationFunctionType.Copy

    nc = bacc.Bacc(None, target_bir_lowering=False)
    xt_d = nc.dram_tensor("xt", [P, IB * BC], f32r, kind="ExternalInput")
    w1_d = nc.dram_tensor("w1", [E, JB, P, IB * P], f32r, kind="ExternalInput")
    b1_d = nc.dram_tensor("b1", [P, E * JB], f32, kind="ExternalInput")
    w2_d = nc.dram_tensor("w2", [P, E * JB], f32r, kind="ExternalInput")
    b2_d = nc.dram_tensor("b2", [1, E], f32, kind="ExternalInput")
    out_d = nc.dram_tensor("out", [1, BC], f32, kind="ExternalOutput")

    with tile.TileContext(nc) as tc:
        with (
            tc.tile_pool(name="const", bufs=1) as const_pool,
            tc.tile_pool(name="xt", bufs=1) as xt_pool,
            tc.tile_pool(name="w1", bufs=4) as w1_pool,
            tc.tile_pool(name="h", bufs=4) as h_pool,
            tc.tile_pool(name="osb", bufs=2) as osb_pool,
            tc.tile_pool(name="fc1ps", bufs=4, space="PSUM") as fc1_psum,
            tc.tile_pool(name="fc2ps", bufs=4, space="PSUM") as fc2_psum,
        ):
            b1_t = const_pool.tile([P, E * JB], f32)
            nc.sync.dma_start(b1_t[:], b1_d[:])
            w2_t = const_pool.tile([P, E * JB], f32r)
            nc.sync.dma_start(w2_t[:], w2_d[:])
            b2_t = const_pool.tile([1, E], f32)
            nc.sync.dma_start(b2_t[:], b2_d[:])
            c_t = const_pool.tile([E, 1], f32)
            nc.any.memset(c_t[:], 1.0 / E)
            o_all = const_pool.tile([E, BC], f32)

            xt_t = xt_pool.tile([P, IB * BC], f32r)
            nc.sync.dma_start(xt_t[:], xt_d[:])

            for e in range(E):
                fc2_ps = [
                    fc2_psum.tile([1, NB], f32, tag="fc2", name=f"fc2ps_{e}_{bh}")
                    for bh in range(BH)
                ]
                for jb in range(JB):
                    w_t = w1_pool.tile([P, IB * P], f32r)
                    nc.sync.dma_start(w_t[:], w1_d[e, jb])
                    col = e * JB + jb
                    for bh in range(BH):
                        ps = fc1_psum.tile([P, NB], f32, tag="fc1")
                        for ib in range(IB):
                            o = ib * BC + bh * NB
                            nc.tensor.matmul(
                                ps[:],
                                w_t[:, ib * P:(ib + 1) * P],
                                xt_t[:, o:o + NB],
                                start=(ib == 0),
                                stop=(ib == IB - 1),
                            )
                        h_t = h_pool.tile([P, NB], f32r)
                        nc.scalar.activation(
                            h_t[:], ps[:], Relu, bias=b1_t[:, col:col + 1]
                        )
                        nc.tensor.matmul(
                            fc2_ps[bh][:],
                            w2_t[:, col:col + 1],
                            h_t[:],
                            start=(jb == 0),
                            stop=(jb == JB - 1),
                        )
                for bh in range(BH):
                    o_stage = osb_pool.tile([1, NB], f32, tag="ostage")
                    nc.scalar.activation(
                        o_stage[:],
                        fc2_ps[bh][:],
                        Sigmoid,
                        bias=b2_t[0:1, e:e + 1],
                    )
                    nc.sync.dma_start(
                        o_all[e:e + 1, bh * NB:(bh + 1) * NB], o_stage[:]
                    )

            for bh in range(BH):
                mps = fc2_psum.tile([1, NB], f32, tag="fc2")
                nc.tensor.matmul(
                    mps[:],
                    c_t[:],
                    o_all[:, bh * NB:(bh + 1) * NB],
                    start=True,
                    stop=True,
                )
                o_sb = osb_pool.tile([1, NB], f32)
                nc.scalar.activation(o_sb[:], mps[:], Copy)
                nc.sync.dma_start(out_d[0:1, bh * NB:(bh + 1) * NB], o_sb[:])
    nc.compile()
    return nc


def prep_inputs(x, W1, b1, W2, b2, E, I, H, BC):
    IB = I // P
    JB = H // P
    w1_l = np.ascontiguousarray(
        W1.reshape(E, IB, P, JB, P).transpose(0, 3, 2, 1, 4).reshape(E, JB, P, IB * P),
        np.float32,
    )
    b1_l = np.ascontiguousarray(
        b1.reshape(E, JB, P).transpose(2, 0, 1).reshape(P, E * JB), np.float32
    )
    w2_l = np.ascontiguousarray(
        W2.reshape(E, JB, P).transpose(2, 0, 1).reshape(P, E * JB), np.float32
    )
    b2_l = np.ascontiguousarray(b2.reshape(1, E), np.float32)
    in_maps = []
    for c in range(N_CORES):
        xc = np.asarray(x[c * BC:(c + 1) * BC], np.float32).T  # [I, BC]
        xt = np.ascontiguousarray(
            xc.reshape(IB, P, BC).transpose(1, 0, 2).reshape(P, IB * BC)
        )
        in_maps.append({"xt": xt, "w1": w1_l, "b1": b1_l, "w2": w2_l, "b2": b2_l})
    return in_maps


def run(x, W1, b1, W2, b2, trace=False):
    E, I, H = W1.shape
    BC = x.shape[0] // N_CORES
    in_maps = prep_inputs(x, W1, b1, W2, b2, E, I, H, BC)
    nc = build_bass(E=E, I=I, H=H, BC=BC)
    res = run_bass_kernel_spmd(nc, in_maps, list(range(N_CORES)), trace=trace)
    outs = [res.results[c]["out"].reshape(BC) for c in range(N_CORES)]
    full = np.concatenate(outs)[:, None].astype(np.float32)
    return full, res


def kernel(x, W1, b1, W2, b2):
    out, _ = run(
        np.asarray(x), np.asarray(W1), np.asarray(b1), np.asarray(W2), np.asarray(b2)
    )
    return out



# revision 11
# speedup vs baseline: 1.3555x; 1.3555x over previous
"""Trainium2 Bass kernel for a 30-expert MLP ensemble.

Computes out[b] = mean_e sigmoid(relu(x @ W1[e] + b1[e]) @ W2[e] + b2[e])
for x [8192, 1024], W1 [30, 1024, 2048], W2 [30, 2048].

Strategy: data-parallel over the batch axis. Each of the 8 cores gets a
1024-row shard of x (pre-transposed on host) and the full replicated
weight stack. No collectives: the host concatenates the 8 disjoint
output shards.

Matmuls run in fp8 (e4m3) with MatmulPerfMode.DoubleRow: two 128-wide
k-subtiles per matmul at 0.5 cycles/output-row, 2x the fp32r/bf16 PE
rate. W1 and W2 are scaled by 64 before the fp8 cast so their U(-1/32..)
values sit in e4m3's normal range; the 1/64 is folded back in via the
activation scale (out = act(psum/64 + bias)). PSUM accumulates fp32, so
the only precision loss is the ~2^-4 fp8 quantization of x, W1, relu(h),
W2 — which averages down over the 1024/2048-long contractions and the
30-expert mean to ~1e-3 relative error (tolerance 2e-2).

Layouts (prepared host-side in numpy, fp8 = ml_dtypes.float8_e4m3):
  xt  [128, IB, BC]      xt[p,s,b]     = x[b, s*128 + p]
  w1  [E*JB, 128, IB, 128] w1[ej,p,s,q] = 64*W1[e, s*128+p, jb*128+q]
  b1  [128, E*JB]        b1[p, e*JB+jb] = b1[e, jb*128 + p]
  w2  [128, E*JB, 1]     w2[p, e*JB+jb] = 64*W2[e, jb*128 + p]
  b2  [1, E]

Per (expert e, hidden pair jp): for each of the two jb in the pair,
4 DoubleRow fc1 matmuls accumulate h^T into PSUM [128, 512] per batch
half; Relu+bias evicts to the fp8 pair tile h[:, u, :]. One DoubleRow
fc2 matmul per (pair, batch half) accumulates the per-expert logits in a
persistent PSUM [1, 512]. Sigmoid+bias writes expert row e of o_all
[30, BC]; a final ones(1/30) fp32 matmul reduces over the expert
partitions to the mean.
"""

import numpy as np

import concourse.bacc as bacc
import concourse.bass as bass
import concourse.mybir as mybir
import concourse.tile as tile
from concourse.bass_utils import run_bass_kernel_spmd

N_CORES = 8
P = 128
NB = 512  # matmul moving free dim (psum bank = 512 fp32)
SCALE = 64.0  # host-side premultiply of W1/W2 before the fp8 cast

E_FULL, I_FULL, H_FULL, B_FULL = 30, 1024, 2048, 8192


def build_bass(E=E_FULL, I=I_FULL, H=H_FULL, BC=B_FULL // N_CORES):
    IB = I // P
    JB = H // P
    BH = BC // NB
    f32 = mybir.dt.float32
    f8 = mybir.dt.float8e4
    DoubleRow = mybir.MatmulPerfMode.DoubleRow
    Relu = mybir.ActivationFunctionType.Relu
    Sigmoid = mybir.ActivationFunctionType.Sigmoid
    Copy = mybir.ActivationFunctionType.Copy

    nc = bacc.Bacc(None, target_bir_lowering=False)
    xt_d = nc.dram_tensor("xt", [P, IB, BC], f8, kind="ExternalInput")
    w1_d = nc.dram_tensor("w1", [E * JB, P, IB, P], f8, kind="ExternalInput")
    b1_d = nc.dram_tensor("b1", [P, E * JB], f32, kind="ExternalInput")
    w2_d = nc.dram_tensor("w2", [P, E * JB], f8, kind="ExternalInput")
    b2_d = nc.dram_tensor("b2", [1, E], f32, kind="ExternalInput")
    out_d = nc.dram_tensor("out", [1, BC], f32, kind="ExternalOutput")

    with tile.TileContext(nc) as tc:
        with (
            tc.tile_pool(name="const", bufs=1) as const_pool,
            tc.tile_pool(name="xt", bufs=1) as xt_pool,
            tc.tile_pool(name="w1", bufs=4) as w1_pool,
            tc.tile_pool(name="h", bufs=4) as h_pool,
            tc.tile_pool(name="osb", bufs=2) as osb_pool,
            tc.tile_pool(name="fc1ps", bufs=4, space="PSUM") as fc1_psum,
            tc.tile_pool(name="fc2ps", bufs=4, space="PSUM") as fc2_psum,
        ):
            b1_t = const_pool.tile([P, E * JB], f32)
            nc.sync.dma_start(b1_t[:], b1_d[:])
            w2_t = const_pool.tile([P, E * JB], f8)
            nc.sync.dma_start(w2_t[:], w2_d[:])
            b2_t = const_pool.tile([1, E], f32)
            nc.sync.dma_start(b2_t[:], b2_d[:])
            c_t = const_pool.tile([E, 1], f32)
            nc.any.memset(c_t[:], 1.0 / E)
            o_all = const_pool.tile([E, BC], f32)

            xt_t = xt_pool.tile([P, IB, BC], f8)
            nc.sync.dma_start(xt_t[:], xt_d[:])

            for e in range(E):
                fc2_ps = [
                    fc2_psum.tile([1, NB], f32, tag="fc2", name=f"fc2ps_{e}_{bh}")
                    for bh in range(BH)
                ]
                for jb in range(JB):
                    col = e * JB + jb
                    w_t = w1_pool.tile([P, IB, P], f8)
                    nc.sync.dma_start(w_t[:], w1_d[col])
                    pss = [
                        fc1_psum.tile(
                            [P, NB], f32, tag="fc1", name=f"fc1ps_{col}_{bh}"
                        )
                        for bh in range(BH)
                    ]
                    for sb in range(0, IB, 2):
                        for bh in range(BH):
                            nc.tensor.matmul(
                                pss[bh][:],
                                w_t[:, sb:sb + 2, :],
                                xt_t[:, sb:sb + 2, bh * NB:(bh + 1) * NB],
                                start=(sb == 0),
                                stop=(sb == IB - 2),
                                perf_mode=DoubleRow,
                            )
                    for bh in range(BH):
                        h_t = h_pool.tile([P, NB], f8, tag="h", name=f"h_{col}_{bh}")
                        nc.scalar.activation(
                            h_t[:],
                            pss[bh][:],
                            Relu,
                            bias=b1_t[:, col:col + 1],
                            scale=1.0 / SCALE,
                        )
                        nc.tensor.matmul(
                            fc2_ps[bh][:],
                            w2_t[:, col:col + 1],
                            h_t[:],
                            start=(jb == 0),
                            stop=(jb == JB - 1),
                        )
                for bh in range(BH):
                    o_stage = osb_pool.tile([1, NB], f32, tag="ostage")
                    nc.scalar.activation(
                        o_stage[:],
                        fc2_ps[bh][0:1, :],
                        Sigmoid,
                        bias=b2_t[0:1, e:e + 1],
                        scale=1.0 / SCALE,
                    )
                    nc.sync.dma_start(
                        o_all[e:e + 1, bh * NB:(bh + 1) * NB], o_stage[:]
                    )

            for bh in range(BH):
                mps = fc2_psum.tile([1, NB], f32, tag="fc2")
                nc.tensor.matmul(
                    mps[:],
                    c_t[:],
                    o_all[:, bh * NB:(bh + 1) * NB],
                    start=True,
                    stop=True,
                )
                o_sb = osb_pool.tile([1, NB], f32)
                nc.scalar.activation(o_sb[:], mps[:], Copy)
                nc.sync.dma_start(out_d[0:1, bh * NB:(bh + 1) * NB], o_sb[:])
    nc.compile()
    return nc


def prep_inputs(x, W1, b1, W2, b2, E, I, H, BC):
    IB = I // P
    JB = H // P
    f8 = mybir.dt.np(mybir.dt.float8e4)
    w1_l = np.ascontiguousarray(
        (W1.astype(np.float32) * SCALE)
        .reshape(E, IB, P, JB, P)
        .transpose(0, 3, 2, 1, 4)
        .reshape(E * JB, P, IB, P)
    ).astype(f8)
    b1_l = np.ascontiguousarray(
        b1.reshape(E, JB, P).transpose(2, 0, 1).reshape(P, E * JB), np.float32
    )
    w2_l = np.ascontiguousarray(
        (W2.astype(np.float32) * SCALE).reshape(E, JB, P).transpose(2, 0, 1)
    ).reshape(P, E * JB).astype(f8)
    b2_l = np.ascontiguousarray(b2.reshape(1, E), np.float32)
    in_maps = []
    for c in range(N_CORES):
        xc = np.asarray(x[c * BC:(c + 1) * BC], np.float32)  # [BC, I]
        xt = np.ascontiguousarray(xc.reshape(BC, IB, P).transpose(2, 1, 0)).astype(f8)
        in_maps.append({"xt": xt, "w1": w1_l, "b1": b1_l, "w2": w2_l, "b2": b2_l})
    return in_maps


def run(x, W1, b1, W2, b2, trace=False):
    E, I, H = W1.shape
    BC = x.shape[0] // N_CORES
    in_maps = prep_inputs(x, W1, b1, W2, b2, E, I, H, BC)
    nc = build_bass(E=E, I=I, H=H, BC=BC)
    res = run_bass_kernel_spmd(nc, in_maps, list(range(N_CORES)), trace=trace)
    outs = [res.results[c]["out"].reshape(BC) for c in range(N_CORES)]
    full = np.concatenate(outs)[:, None].astype(np.float32)
    return full, res


def kernel(x, W1, b1, W2, b2):
    out, _ = run(
        np.asarray(x), np.asarray(W1), np.asarray(b1), np.asarray(W2), np.asarray(b2)
    )
    return out


# revision 13
# speedup vs baseline: 2.1382x; 1.5775x over previous
"""Trainium2 Bass kernel for a 30-expert MLP ensemble.

Computes out[b] = mean_e sigmoid(relu(x @ W1[e] + b1[e]) @ W2[e] + b2[e])
for x [8192, 1024], W1 [30, 1024, 2048], W2 [30, 2048].

Strategy: data-parallel over the batch axis. Each of the 8 cores gets a
1024-row shard of x (pre-transposed on host) and the full replicated
weight stack. No collectives: the host concatenates the 8 disjoint
output shards.

All matmuls run in fp8 (e4m3) with MatmulPerfMode.DoubleRow: two
128-wide k-subtiles per matmul at 2 moving-rows/cycle, 2x the
fp32r/bf16 PE rate. W1 and W2 are scaled by 64 before the fp8 cast so
their U(-1/32..) values sit in e4m3's normal range; the 1/64 is folded
back in via the activation scale (out = act(psum/64 + bias)). PSUM
accumulates fp32, so the only precision loss is the ~2^-4 fp8
quantization of x, W1, relu(h), W2 — which averages down over the
1024/2048-long contractions and the 30-expert mean to ~1e-3 relative
error (tolerance 2e-2).

fc2 contracts hidden pairs (2jp, 2jp+1) per DoubleRow matmul. Its
stationary is w2 replicated across all 128 PE columns (the ldweights
ISA rejects narrow DoubleRow stationaries; 128 columns matches the
known-good fc1 shape), so all 128 PSUM partitions hold the same logit
row and sigmoid reads partition 0. The fc2 matmul for pair step N is
emitted after the fc1 group of step N+1, giving the Relu eviction a
full fc1 group (~2us) of slack so the PE never stalls on the Scalar
engine (stalls also drop the PE out of its max p-state).

Layouts (prepared host-side in numpy, fp8 = ml_dtypes.float8_e4m3):
  xt  [128, IB, BC]        xt[p,s,b]      = x[b, s*128 + p]
  w1  [E*JB, 128, IB, 128] w1[ej,p,s,q]   = 64*W1[e, s*128+p, jb*128+q]
  b1  [128, E*JB]          b1[p, e*JB+jb] = b1[e, jb*128 + p]
  w2r [128, E*JP, 2, 128]  w2r[p,gp,u,q]  = 64*W2[e, (2jp+u)*128+p]
  b2  [1, E]
"""

import numpy as np

import concourse.bacc as bacc
import concourse.bass as bass
import concourse.mybir as mybir
import concourse.tile as tile
from concourse.bass_utils import run_bass_kernel_spmd

N_CORES = 8
P = 128
NB = 512  # matmul moving free dim (psum bank = 512 fp32)
SCALE = 64.0  # host-side premultiply of W1/W2 before the fp8 cast

E_FULL, I_FULL, H_FULL, B_FULL = 30, 1024, 2048, 8192


def build_bass(E=E_FULL, I=I_FULL, H=H_FULL, BC=B_FULL // N_CORES):
    IB = I // P
    JB = H // P
    JP = JB // 2
    BH = BC // NB
    f32 = mybir.dt.float32
    f8 = mybir.dt.float8e4
    DoubleRow = mybir.MatmulPerfMode.DoubleRow
    Relu = mybir.ActivationFunctionType.Relu
    Sigmoid = mybir.ActivationFunctionType.Sigmoid
    Copy = mybir.ActivationFunctionType.Copy

    nc = bacc.Bacc(None, target_bir_lowering=False)
    xt_d = nc.dram_tensor("xt", [P, IB, BC], f8, kind="ExternalInput")
    w1_d = nc.dram_tensor("w1", [E * JB, P, IB, P], f8, kind="ExternalInput")
    b1_d = nc.dram_tensor("b1", [P, E * JB], f32, kind="ExternalInput")
    w2_d = nc.dram_tensor("w2", [P, E * JP, 2, P], f8, kind="ExternalInput")
    b2_d = nc.dram_tensor("b2", [1, E], f32, kind="ExternalInput")
    out_d = nc.dram_tensor("out", [1, BC], f32, kind="ExternalOutput")

    with tile.TileContext(nc) as tc:
        with (
            tc.tile_pool(name="const", bufs=1) as const_pool,
            tc.tile_pool(name="xt", bufs=1) as xt_pool,
            tc.tile_pool(name="w1", bufs=4) as w1_pool,
            tc.tile_pool(name="h", bufs=4) as h_pool,
            tc.tile_pool(name="osb", bufs=2) as osb_pool,
            tc.tile_pool(name="fc1ps", bufs=4, space="PSUM") as fc1_psum,
            tc.tile_pool(name="fc2ps", bufs=4, space="PSUM") as fc2_psum,
        ):
            b1_t = const_pool.tile([P, E * JB], f32)
            nc.sync.dma_start(b1_t[:], b1_d[:])
            w2_t = const_pool.tile([P, E * JP, 2, P], f8)
            nc.sync.dma_start(w2_t[:], w2_d[:])
            b2_t = const_pool.tile([1, E], f32)
            nc.sync.dma_start(b2_t[:], b2_d[:])
            c_t = const_pool.tile([E, 1], f32)
            nc.any.memset(c_t[:], 1.0 / E)
            o_all = const_pool.tile([E, BC], f32)

            xt_t = xt_pool.tile([P, IB, BC], f8)
            nc.sync.dma_start(xt_t[:], xt_d[:])

            fc2_ps_by_e = {}

            def fc1_step(gp):
                """fc1 for hidden pair gp: 8 DoubleRow matmuls + 4 Relu
                evictions into the fp8 pair tiles h_ts[bh][:, u, :]."""
                e, jp = divmod(gp, JP)
                if jp == 0:
                    fc2_ps_by_e[e] = [
                        fc2_psum.tile(
                            [P, NB], f32, tag="fc2", name=f"fc2ps_{e}_{bh}"
                        )
                        for bh in range(BH)
                    ]
                h_ts = [
                    h_pool.tile([P, 2, NB], f8, tag="h", name=f"h_{gp}_{bh}")
                    for bh in range(BH)
                ]
                for u in range(2):
                    jb = 2 * jp + u
                    col = e * JB + jb
                    w_t = w1_pool.tile([P, IB, P], f8, tag="w1", name=f"w1_{col}")
                    nc.sync.dma_start(w_t[:], w1_d[col])
                    pss = [
                        fc1_psum.tile(
                            [P, NB], f32, tag="fc1", name=f"fc1ps_{col}_{bh}"
                        )
                        for bh in range(BH)
                    ]
                    for sb in range(0, IB, 2):
                        for bh in range(BH):
                            nc.tensor.matmul(
                                pss[bh][:],
                                w_t[:, sb:sb + 2, :],
                                xt_t[:, sb:sb + 2, bh * NB:(bh + 1) * NB],
                                start=(sb == 0),
                                stop=(sb == IB - 2),
                                perf_mode=DoubleRow,
                            )
                    for bh in range(BH):
                        nc.scalar.activation(
                            h_ts[bh][:, u, :],
                            pss[bh][:],
                            Relu,
                            bias=b1_t[:, col:col + 1],
                            scale=1.0 / SCALE,
                        )
                return (e, jp, h_ts)

            def fc2_step(st):
                e, jp, h_ts = st
                for bh in range(BH):
                    nc.tensor.matmul(
                        fc2_ps_by_e[e][bh][:],
                        w2_t[:, e * JP + jp, :, :],
                        h_ts[bh][:],
                        start=(jp == 0),
                        stop=(jp == JP - 1),
                        perf_mode=DoubleRow,
                    )

            def sig_step(e):
                for bh in range(BH):
                    o_stage = osb_pool.tile(
                        [1, NB], f32, tag="ostage", name=f"osig_{e}_{bh}"
                    )
                    nc.scalar.activation(
                        o_stage[:],
                        fc2_ps_by_e[e][bh][0:1, :],
                        Sigmoid,
                        bias=b2_t[0:1, e:e + 1],
                        scale=1.0 / SCALE,
                    )
                    nc.sync.dma_start(
                        o_all[e:e + 1, bh * NB:(bh + 1) * NB], o_stage[:]
                    )
                del fc2_ps_by_e[e]

            prev = None
            for gp in range(E * JP):
                st = fc1_step(gp)
                if prev is not None:
                    fc2_step(prev)
                    if prev[1] == JP - 1:
                        sig_step(prev[0])
                prev = st
            fc2_step(prev)
            sig_step(prev[0])

            for bh in range(BH):
                mps = fc2_psum.tile([1, NB], f32, tag="fc2")
                nc.tensor.matmul(
                    mps[:],
                    c_t[:],
                    o_all[:, bh * NB:(bh + 1) * NB],
                    start=True,
                    stop=True,
                )
                o_sb = osb_pool.tile([1, NB], f32)
                nc.scalar.activation(o_sb[:], mps[:], Copy)
                nc.sync.dma_start(out_d[0:1, bh * NB:(bh + 1) * NB], o_sb[:])
    nc.compile()
    return nc


def prep_inputs(x, W1, b1, W2, b2, E, I, H, BC):
    IB = I // P
    JB = H // P
    JP = JB // 2
    f8 = mybir.dt.np(mybir.dt.float8e4)
    w1_l = np.ascontiguousarray(
        (W1.astype(np.float32) * SCALE)
        .reshape(E, IB, P, JB, P)
        .transpose(0, 3, 2, 1, 4)
        .reshape(E * JB, P, IB, P)
    ).astype(f8)
    b1_l = np.ascontiguousarray(
        b1.reshape(E, JB, P).transpose(2, 0, 1).reshape(P, E * JB), np.float32
    )
    # w2 replicated across all 128 stationary columns (see module docstring)
    w2_n = (
        (W2.astype(np.float32) * SCALE)
        .reshape(E, JP, 2, P)
        .transpose(3, 0, 1, 2)
        .reshape(P, E * JP, 2, 1)
        .astype(f8)
    )
    w2_l = np.ascontiguousarray(np.broadcast_to(w2_n, (P, E * JP, 2, P)))
    b2_l = np.ascontiguousarray(b2.reshape(1, E), np.float32)
    in_maps = []
    for c in range(N_CORES):
        xc = np.asarray(x[c * BC:(c + 1) * BC], np.float32)  # [BC, I]
        xt = np.ascontiguousarray(xc.reshape(BC, IB, P).transpose(2, 1, 0)).astype(f8)
        in_maps.append({"xt": xt, "w1": w1_l, "b1": b1_l, "w2": w2_l, "b2": b2_l})
    return in_maps


def run(x, W1, b1, W2, b2, trace=False):
    E, I, H = W1.shape
    BC = x.shape[0] // N_CORES
    in_maps = prep_inputs(x, W1, b1, W2, b2, E, I, H, BC)
    nc = build_bass(E=E, I=I, H=H, BC=BC)
    res = run_bass_kernel_spmd(nc, in_maps, list(range(N_CORES)), trace=trace)
    outs = [res.results[c]["out"].reshape(BC) for c in range(N_CORES)]
    full = np.concatenate(outs)[:, None].astype(np.float32)
    return full, res


def kernel(x, W1, b1, W2, b2):
    out, _ = run(
        np.asarray(x), np.asarray(W1), np.asarray(b1), np.asarray(W2), np.asarray(b2)
    )
    return out
